# revision 1
# baseline (speedup 1.0000x reference)
"""v15: v3 weave + software-pipelined attnV + PE-side causal mask
+ bf16 transpose path.

- attn@V for key tile c is emitted one step behind the score/exp pair of
  tile c+1, so the in-order PE queue never stalls waiting for an exp it
  could have overlapped with the next score matmuls.
- the causal mask on diagonal 128-blocks is applied by accumulating a
  constant -240 upper-triangle into the score PSUM (one extra 128-wide
  matmul, start=False) BEFORE exp, instead of a post-exp DVE tensor_mul.
- inputs are cast to bf16 on the idle Pool engine before the PE
  transposes (1 c/row instead of 2 for f32); Q/K/V projections then run
  bf16. Transpose output lands in a bf16 bitcast view of the f32 PSUM
  tile (no extra banks); scores/out-proj stay f32r as before.
Measured: ~154us test.py slope (v13: 206, v9: 296.3, v3: 314.0),
rel err 4.4e-3.
"""

import numpy as np

B = 8
L = 2048
D = 512
H = 8
DH = 64
NT = L // 128
NCH = D // 128
NQ = L // 512

_cached = {}


def _build(repeat=1, cfg=None):
    cfg = dict(cfg or {})
    PS512 = cfg.get("ps512", 2)
    SPS2 = cfg.get("sps2", 2)
    OPS = cfg.get("ops", 2)
    PEXP = cfg.get("pexp", 6)
    OSB = cfg.get("osb", 3)
    XIN = cfg.get("xin", 8)
    XT = cfg.get("xt", 10)
    import concourse.tile as tile
    from concourse import mybir, bacc
    from concourse.masks import make_identity

    f32 = mybir.dt.float32
    bf16 = mybir.dt.bfloat16
    f32r = mybir.dt.float32r

    nc = bacc.Bacc("TRN2", target_bir_lowering=False, debug=False)

    xq = nc.dram_tensor("query", [L, D], f32, kind="ExternalInput").ap()
    xk = nc.dram_tensor("key", [L, D], f32, kind="ExternalInput").ap()
    xv = nc.dram_tensor("value", [L, D], f32, kind="ExternalInput").ap()
    Wq = nc.dram_tensor("Wq", [D, D], f32, kind="ExternalInput").ap()
    Wk = nc.dram_tensor("Wk", [D, D], f32, kind="ExternalInput").ap()
    Wv = nc.dram_tensor("Wv", [D, D], f32, kind="ExternalInput").ap()
    Wo = nc.dram_tensor("Wo", [D, D], f32, kind="ExternalInput").ap()
    bq = nc.dram_tensor("bq", [D], f32, kind="ExternalInput").ap()
    bk = nc.dram_tensor("bk", [D], f32, kind="ExternalInput").ap()
    bv = nc.dram_tensor("bv", [D], f32, kind="ExternalInput").ap()
    bo = nc.dram_tensor("bo", [D], f32, kind="ExternalInput").ap()
    out = nc.dram_tensor("out", [L, D], f32, kind="ExternalOutput").ap()

    def r(ap):
        return ap.bitcast(f32r)

    with tile.TileContext(nc) as tc:
        with (
            tc.tile_pool(name="persist", bufs=1) as persist,
            tc.tile_pool(name="consts", bufs=1) as consts,
            tc.tile_pool(name="ps512", bufs=PS512, space="PSUM") as ps512,
            tc.tile_pool(name="sps2", bufs=SPS2, space="PSUM") as sps2_pool,
            tc.tile_pool(name="ops", bufs=OPS, space="PSUM") as ops_pool,
        ):
            # ---- constants ----
            ident = consts.tile([128, 128], f32, tag="ident")
            make_identity(nc, ident[:])
            # maskU[p, c] = -240 where p > c (anti-causal), 0 elsewhere;
            # added to the diagonal score block pre-exp so exp gives ~1e-13
            maskU = consts.tile([128, 128], bf16, tag="maskU")
            nc.gpsimd.memset(maskU[:], -240.0)
            nc.gpsimd.affine_select(
                out=maskU[:], in_=maskU[:], compare_op=mybir.AluOpType.is_gt,
                fill=0.0, base=0, pattern=[[-1, 128]], channel_multiplier=1,
            )
            identb = consts.tile([128, 128], bf16, tag="identb")
            nc.vector.tensor_copy(identb[:], ident[:])
            ones = consts.tile([1, 512], f32, tag="ones")
            nc.vector.memset(ones[:], 1.0)
            ones_t = consts.tile([128, 64], bf16, tag="ones_t")
            nc.vector.memset(ones_t[:], 1.0)

            # ---- weights / biases ----
            w_sb = {}
            b_row = {}
            with tc.tile_pool(name="wtmp", bufs=3) as wtmp_pool:
                for name, wdram in (("q", Wq), ("k", Wk), ("v", Wv), ("o", Wo)):
                    dt = f32 if name == "o" else bf16
                    t = persist.tile([128, NCH, 512], dt, tag=f"W{name}",
                                     name=f"W{name}")
                    for c in range(NCH):
                        wt = wtmp_pool.tile([128, 512], f32, tag="wtmp",
                                            name="wtmp")
                        nc.gpsimd.dma_start(
                            wt[:], wdram[128 * c:128 * (c + 1), :])
                        if name == "o":
                            nc.vector.tensor_copy(r(t[:, c, :]), wt[:])
                        else:
                            nc.vector.tensor_copy(t[:, c, :], wt[:])
                    w_sb[name] = t
                for name, bdram in (("q", bq), ("k", bk), ("v", bv), ("o", bo)):
                    t = wtmp_pool.tile([1, 512], f32, tag=f"b{name}",
                                       name=f"b{name}", bufs=1)
                    nc.gpsimd.dma_start(t[:], bdram[None, :])
                    b_row[name] = t
                # per-partition bias columns for q/k (dout on partitions)
                bcol = {}
                for name in ("q", "k"):
                    bc_t = consts.tile([128, NCH], f32, tag=f"bcol{name}",
                                       name=f"bcol{name}")
                    for c in range(NCH):
                        tp = ps512.tile([128, 512], f32, tag="ps512", name="ps512")
                        nc.tensor.transpose(
                            tp[:, 0:1], b_row[name][0:1, 128 * c:128 * (c + 1)],
                            ident[0:1, 0:1])
                        nc.vector.tensor_copy(bc_t[:, c:c + 1], tp[:, 0:1])
                    bcol[name] = bc_t
                # broadcast bias tiles for v (head-interleaved) and o (natural)
                bvb = consts.tile([128, H, DH], f32, tag="bvb", name="bvb")
                bob = consts.tile([128, 512], f32, tag="bob", name="bob")
                for dst, row in ((bvb, b_row["v"]), (bob, b_row["o"])):
                    rowr = wtmp_pool.tile([1, 512], f32, tag="browr",
                                          name="browr", bufs=2)
                    nc.vector.tensor_copy(r(rowr[:]), row[:])
                    tp = ps512.tile([128, 512], f32, tag="ps512", name="ps512")
                    nc.tensor.matmul(tp[:], r(ones[0:1, 0:128]), r(rowr[:]),
                                     start=True, stop=True)
                    if dst is bvb:
                        nc.vector.tensor_copy(
                            dst[:], tp[:].rearrange("p (h d) -> p h d", h=H))
                    else:
                        nc.vector.tensor_copy(dst[:], tp[:])

            # ---- persistent activations ----
            kt_sb = [persist.tile([128, L], f32, tag=f"KT{c}", name=f"KT{c}")
                     for c in range(NCH)]
            v_sb = [persist.tile([128, H, DH + 1], bf16, tag=f"V{t}",
                        name=f"V{t}") for t in range(NT)]
            stage = [persist.tile([128, L], f32, tag=f"stage{c}", name=f"stage{c}")
                     for c in range(NCH)]

            with (
                tc.tile_pool(name="xin", bufs=XIN) as xin_pool,
                tc.tile_pool(name="qtg", bufs=2) as qtg_pool,
                tc.tile_pool(name="xt", bufs=XT) as xt_pool,
                tc.tile_pool(name="pexp", bufs=PEXP) as p_pool,
                tc.tile_pool(name="norm", bufs=1) as norm_pool,
                tc.tile_pool(name="osb", bufs=OSB) as o_pool,
            ):
                def emit_a_pieces(g):
                    qt_g = [qtg_pool.tile([128, 512], f32, tag=f"qtg{c}",
                                          name=f"qtg{c}") for c in range(NCH)]
                    pieces = []
                    state = {}
                    for tname_, xdram_ in (("k", xk), ("v", xv), ("q", xq)):
                        pieces.append(
                            lambda tname=tname_, xdram=xdram_:
                            state.__setitem__(
                                tname, emit_a_transpose(g, xdram)))
                        pieces.append(
                            lambda tname=tname_: emit_a_proj(
                                g, tname, state[tname], qt_g))
                    return qt_g, pieces

                def emit_a_transpose(g, xdram):
                    if True:
                        xtiles = []
                        for j in range(4):
                            t0 = 4 * g + j
                            xt_in = xin_pool.tile([128, 512], f32, tag="xin",
                                                  name="xin")
                            nc.sync.dma_start(
                                xt_in[:], xdram[128 * t0:128 * (t0 + 1), :])
                            # cast to bf16 on the idle Pool engine so the
                            # transposes run at 1 c/row instead of 2
                            xb = xt_pool.tile([128, 512], bf16, tag="xb",
                                              name="xb", bufs=6)
                            nc.gpsimd.tensor_copy(xb[:], xt_in[:])
                            xtiles.append(xb)
                        xt_c = []
                        for c in range(NCH):
                            ps = ps512.tile([128, 512], f32, tag="ps512",
                                            name="ps512")
                            psb = ps[:, 0:256].bitcast(bf16)
                            for j in range(4):
                                nc.tensor.transpose(
                                    psb[:, 128 * j:128 * (j + 1)],
                                    xtiles[j][:, 128 * c:128 * (c + 1)],
                                    identb[:],
                                )
                            sb = xt_pool.tile([128, 512], bf16, tag="xt",
                                              name="xt")
                            nc.vector.tensor_copy(sb[:], psb)
                            xt_c.append(sb)
                        return xt_c

                def emit_a_proj(g, tname, xt_c, qt_g):
                    if True:
                        if tname in ("q", "k"):
                            for co in range(NCH):
                                pp = ps512.tile([128, 512], f32, tag="ps512",
                                                name="ps512")
                                for ci in range(NCH):
                                    nc.tensor.matmul(
                                        pp[:],
                                        w_sb[tname][
                                            :, ci, 128 * co:128 * (co + 1)],
                                        xt_c[ci][:],
                                        start=(ci == 0), stop=(ci == NCH - 1),
                                    )
                                if tname == "q":
                                    nc.vector.tensor_scalar_add(
                                        r(qt_g[co][:]), pp[:],
                                        bcol["q"][:, co:co + 1])
                                else:
                                    nc.vector.tensor_scalar_add(
                                        r(kt_sb[co][:, 512 * g:512 * (g + 1)]),
                                        pp[:], bcol["k"][:, co:co + 1])
                        else:
                            for j in range(4):
                                t0 = 4 * g + j
                                pv = ps512.tile([128, 512], f32, tag="ps512",
                                                name="ps512")
                                for ci in range(NCH):
                                    nc.tensor.matmul(
                                        pv[:],
                                        xt_c[ci][:, 128 * j:128 * (j + 1)],
                                        w_sb["v"][:, ci, :],
                                        start=(ci == 0), stop=(ci == NCH - 1),
                                    )
                                nc.vector.tensor_add(
                                    v_sb[t0][:, :, 0:DH],
                                    pv[:].rearrange("p (h d) -> p h d", h=H),
                                    bvb[:],
                                )
                                nc.gpsimd.memset(v_sb[t0][:, :, DH:DH + 1], 1.0)

                def emit_b_qt(qt, qt_g, weave=()):
                    weave = list(weave)
                    kmax = 4 * qt + 4
                    stg = norm_pool.tile([128, 1536], f32, tag="stg", name="stg")
                    for hp in range(H // 2):
                        ch = hp
                        kth = kt_sb[ch]
                        qth = qt_g[ch]
                        po = [ops_pool.tile([65, 512], f32, tag="ops",
                                            name="ops") for _ in range(2)]

                        def emit_av(c, pt):
                            m = c - 4 * qt
                            jv0 = 0 if m < 1 else 128 * m
                            for k in range(2):
                                nc.tensor.matmul(
                                    po[k][:, jv0:512],
                                    v_sb[c][:, 2 * hp + k, :],
                                    pt[:, 512 * k + jv0:512 * (k + 1)],
                                    start=(c == 0), stop=(c == kmax - 1),
                                )

                        pending = None
                        for c in range(kmax):
                            m = c - 4 * qt
                            js0 = 0 if m < 1 else (128 * m if m < 3 else 256)
                            jv0 = 0 if m < 1 else 128 * m
                            ps = sps2_pool.tile([128, 1024], f32, tag="sps2",
                                                name="sps2")
                            pt = p_pool.tile([128, 1024], bf16, tag="pexp",
                                             name="pexp")
                            for k in range(2):
                                prow = 64 * k
                                nc.tensor.matmul(
                                    ps[:, 512 * k + js0:512 * (k + 1)],
                                    r(kth[prow:prow + DH,
                                          128 * c:128 * (c + 1)]),
                                    r(qth[prow:prow + DH, js0:512]),
                                    start=True, stop=(m < 0),
                                )
                            if m < 0:
                                nc.scalar.activation(
                                    pt[:], ps[:],
                                    mybir.ActivationFunctionType.Exp,
                                    scale=0.125,
                                )
                            else:
                                # accumulate -240 above the diagonal of the
                                # 128-wide diag block (PE, no DVE hop)
                                for k in range(2):
                                    nc.tensor.matmul(
                                        ps[:, 512 * k + 128 * m:
                                           512 * k + 128 * (m + 1)],
                                        identb[:], maskU[:],
                                        start=False, stop=True,
                                        skip_group_check=True,
                                    )
                                for k in range(2):
                                    nc.scalar.activation(
                                        pt[:, 512 * k + jv0:512 * (k + 1)],
                                        ps[:, 512 * k + jv0:512 * (k + 1)],
                                        mybir.ActivationFunctionType.Exp,
                                        scale=0.125,
                                    )
                            if pending is not None:
                                emit_av(*pending)
                            pending = (c, pt)
                        emit_av(*pending)
                        for k in range(2):
                            h = 2 * hp + k
                            prow = 64 * k
                            nc.vector.tensor_copy(
                                r(stage[ch][prow:prow + DH,
                                            512 * qt:512 * (qt + 1)]),
                                po[k][0:DH, :])
                            nc.vector.tensor_copy(
                                stg[32 * (h % 3):32 * (h % 3) + 1,
                                    512 * (h // 3):512 * (h // 3) + 512],
                                po[k][DH:DH + 1, :])
                        if weave and hp >= 1:
                            weave.pop(0)()
                            if weave:
                                weave.pop(0)()
                    rstg = norm_pool.tile([128, 1536], f32, tag="rstg",
                                          name="rstg")
                    nc.vector.reciprocal_approx_fast(out=rstg[:], in_=stg[:])
                    rbf = norm_pool.tile([128, 1536], bf16, tag="rbf", name="rbf")
                    nc.vector.tensor_copy(rbf[:], rstg[:])
                    for ch in range(NCH):
                        bcp = ps512.tile([128, 512], f32, tag="ps512",
                                         name="ps512")
                        for sub in range(2):
                            hh = 2 * ch + sub
                            pp0 = 32 * (hh % 3)
                            fo = 512 * (hh // 3)
                            nc.tensor.matmul(
                                bcp[64 * sub:64 * sub + 64, :],
                                ones_t[pp0:pp0 + 1, 0:64],
                                rbf[pp0:pp0 + 1, fo:fo + 512],
                                start=True, stop=True,
                            )
                        nc.vector.tensor_mul(
                            r(stage[ch][:, 512 * qt:512 * (qt + 1)]),
                            stage[ch][:, 512 * qt:512 * (qt + 1)],
                            bcp[:],
                        )
                    for i in range(4 * qt, 4 * qt + 4):
                        pout = ps512.tile([128, 512], f32, tag="ps512",
                                          name="ps512")
                        for ch in range(NCH):
                            nc.tensor.matmul(
                                pout[:],
                                r(stage[ch][:, 128 * i:128 * (i + 1)]),
                                r(w_sb["o"][:, ch, :]),
                                start=(ch == 0), stop=(ch == NCH - 1),
                            )
                        ot = o_pool.tile([128, 512], f32, tag="osb", name="osb")
                        nc.vector.tensor_add(ot[:], pout[:], bob[:])
                        nc.sync.dma_start(out[128 * i:128 * (i + 1), :], ot[:])
                    for w in weave:
                        w()

                def emit_body():
                    qt_g, pieces = emit_a_pieces(0)
                    for p in pieces:
                        p()
                    for g in range(NQ):
                        if g + 1 < NQ:
                            qt_next, weave = emit_a_pieces(g + 1)
                        else:
                            qt_next, weave = None, ()
                        emit_b_qt(g, qt_g, weave)
                        qt_g = qt_next

                if repeat > 1:
                    with tc.For_i(0, repeat, 1, hint_engines=(
                            mybir.EngineType.PE,
                            mybir.EngineType.DVE,
                            mybir.EngineType.Activation,
                            mybir.EngineType.SP,
                            mybir.EngineType.Pool)):
                        emit_body()
                else:
                    emit_body()

    nc.compile()
    return nc


def get_nc(repeat=1, cfg=None):
    key = f"nc{repeat}-{sorted((cfg or {}).items())}"
    if key not in _cached:
        _cached[key] = _build(repeat, cfg)
    return _cached[key]


def run(in_maps, trace=False, repeat=1, cfg=None, **kw):
    from concourse.bass_utils import run_bass_kernel_spmd

    nc = get_nc(repeat, cfg)
    return run_bass_kernel_spmd(nc, in_maps, list(range(B)), trace=trace, **kw)


def kernel(query, key, value, Wq, bq, Wk, bk, Wv, bv, Wo, bo):
    shared = {
        "Wq": np.ascontiguousarray(Wq, np.float32),
        "Wk": np.ascontiguousarray(Wk, np.float32),
        "Wv": np.ascontiguousarray(Wv, np.float32),
        "Wo": np.ascontiguousarray(Wo, np.float32),
        "bq": np.ascontiguousarray(bq, np.float32),
        "bk": np.ascontiguousarray(bk, np.float32),
        "bv": np.ascontiguousarray(bv, np.float32),
        "bo": np.ascontiguousarray(bo, np.float32),
    }
    in_maps = []
    for i in range(B):
        m = dict(shared)
        m["query"] = np.ascontiguousarray(query[i], np.float32)
        m["key"] = np.ascontiguousarray(key[i], np.float32)
        m["value"] = np.ascontiguousarray(value[i], np.float32)
        in_maps.append(m)
    res = run(in_maps)
    return np.stack([res.results[i]["out"] for i in range(B)], axis=0)

